# revision 21
# baseline (speedup 1.0000x reference)
"""Trainium2 Bass kernel for nn_Network_58987080843722 (gnn_message_passing).

Computation (per batch element b, record r = indices[b]):
  1. Inverse-square-distance interpolation of code vectors at 8192 query
     points against 128 codes:  w_k(p) ~ 1/|q_p - c_k|^2 (normalized),
     query_codes = sum_k w_k * codes[r,k]                       (128-dim)
  2. 6-layer MLP with skip concats of x = [query_codes, q] (131 in-dims).

Sharding: pure data-parallel - core b handles batch element b (B=8 = 8
cores), decoder weights replicated, codes/codes_position gathered on host
(only 8 of 4096 records are touched).

Device mapping notes:
  * Interpolation runs points-on-partitions in exact fp32 (Square-with-bias
    on ACT), normalized weights transposed to codes-on-partitions via PE.
  * All query-code skip terms are folded host-side into per-record
    codes-space chunks M_l = bc @ W_l[qc-part], so the MLP never needs an
    explicit query_codes tensor: z_l's qc contribution = M_l^T w.
  * The rank-3 query-point skip terms of L2..L5 are computed OFF the PE:
    three DVE scalar_tensor_tensor ops accumulate  z += qp_a * Wq[a,:]
    into the stopped PSUM bank (qp broadcast to 128 partitions via DMA,
    per-output-feature weight columns as per-partition scalar APs).  L1's
    qp term stays on the PE (its PSUM groups are too short to tolerate
    engine round-trips).
  * PSUM->SBUF drains apply leaky-relu in a single ACT op (Lrelu, alpha);
    during L1 (short groups) the drain is rate-limited on ACT, so a few
    L1 drains are routed to DVE/Pool as copy+max pairs.
"""

import numpy as np

import concourse.bass as bass
import concourse.mybir as mybir
import concourse.tile as tile
from concourse import bacc
from concourse.bass import ds, ts
from concourse.bass_utils import run_bass_kernel_spmd
from concourse.masks import make_identity

f32 = mybir.dt.float32
f16 = mybir.dt.float16

B, P, K, D = 8, 8192, 128, 128
TN = 512                   # points per chunk (matmul moving dim)
NCHUNK = P // TN           # 16
NPT = TN // 128            # 4 point-tiles (128 points each) per chunk
NTILE_P = P // 128         # 64 point-tiles total

# per layer: (n_h_chunks, n_out_tiles_of_128); h-chunks exclude the folded
# codes-space chunk (w-chunk) and the rank-3 qp term.
LAYERS = [(0, 16), (16, 8), (8, 4), (4, 2), (2, 1)]

# L1 drain routing: how many of the 16 L1 z-tiles drain via DVE / Pool
# (copy+max pairs) instead of single-op ACT Lrelu.
L1_DVE_DRAINS = 0
L1_POOL_DRAINS = 0

_BUILT = None
REPEAT = 1  # >1: repeat the whole computation (timing calibration only)



def _build():
    """Build + compile the SPMD Bass module (identical program on 8 cores)."""
    nc = bacc.Bacc(
        "TRN2",
        target_bir_lowering=False,
        debug=False,
        enable_asserts=False,
        num_devices=8,
    )

    qptn_d = nc.dram_tensor("qptn", [128, NTILE_P, 3], f32, kind="ExternalInput")
    qp3_d = nc.dram_tensor("qp3", [3, P], f16, kind="ExternalInput")
    qpb_d = nc.dram_tensor("qpb", [128, NCHUNK, 3, TN], f16, kind="ExternalInput")
    cb_d = nc.dram_tensor("cb", [128, 3, K], f32, kind="ExternalInput")
    wt0_d = nc.dram_tensor("wt0", [128, NPT, 128], f16, kind="ExternalInput")
    wh_d, wm_d, wqc_d, wqp_d = {}, {}, {}, {}
    for i, (nh, nt) in enumerate(LAYERS):
        li = i + 1
        if nh:
            wh_d[li] = nc.dram_tensor(f"wh{li}", [128, nh, nt, 128], f16,
                                      kind="ExternalInput")
        wm_d[li] = nc.dram_tensor(f"wm{li}", [128, nt, 128], f16,
                                  kind="ExternalInput")
        if li in (2, 3):
            wqc_d[li] = nc.dram_tensor(f"wqc{li}", [128, 3, nt], f32,
                                       kind="ExternalInput")
        if li >= 2:
            wqp_d[li] = nc.dram_tensor(f"wqp{li}", [3, nt, 128], f16,
                                       kind="ExternalInput")
    wq1_d = nc.dram_tensor("wq1", [3, 16, 128], f16, kind="ExternalInput")
    w6_d = nc.dram_tensor("w6", [128, 1], f16, kind="ExternalInput")
    out_d = nc.dram_tensor("out", [1, P], f32, kind="ExternalOutput")

    AF = mybir.ActivationFunctionType
    OP = mybir.AluOpType

    with tile.TileContext(nc) as tc:
        with (
            tc.tile_pool(name="const", bufs=1) as cpool,
            tc.tile_pool(name="work", bufs=4) as wpool,
            tc.tile_pool(name="qpbp", bufs=2) as qpbp,
            tc.tile_pool(name="hpool", bufs=1) as hpool,
            tc.tile_pool(name="psZ", bufs=6, space=bass.MemorySpace.PSUM) as psZ,
            tc.tile_pool(name="psI", bufs=1, space=bass.MemorySpace.PSUM) as psI,
            tc.tile_pool(name="psS", bufs=1, space=bass.MemorySpace.PSUM) as psS,
        ):
            ident = cpool.tile([128, 128], f16)
            make_identity(nc, ident[:])

            # sync-queue DMAs ordered by first use: chunk-0 L1 needs wt0 +
            # qp3c immediately; interp(1) needs qptn/cbt only mid-chunk-0.
            wT0 = cpool.tile([128, NPT, 128], f16, tag="wT0")
            nc.sync.dma_start(wT0[:], wt0_d[:])
            qp3c = cpool.tile([3, P], f16)
            nc.sync.dma_start(qp3c[:], qp3_d[:])
            qpb0 = qpbp.tile([128, 3, TN], f16, tag="qpb")
            nc.sync.dma_start(qpb0[:], qpb_d[:, 0, :, :])
            qptn = cpool.tile([128, NTILE_P, 3], f32)
            nc.sync.dma_start(qptn[:], qptn_d[:])
            cbt = cpool.tile([128, 3, K], f32)
            nc.sync.dma_start(cbt[:], cb_d[:])
            wq1 = cpool.tile([3, 16, 128], f16)
            nc.gpsimd.dma_start(wq1[:], wq1_d[:])
            whs, wms, wqcs, wqps = {}, {}, {}, {}
            for i, (nh, nt) in enumerate(LAYERS):
                li = i + 1
                tm = cpool.tile([128, nt, 128], f16, tag=f"wm{li}")
                nc.gpsimd.dma_start(tm[:], wm_d[li][:])
                wms[li] = tm
                if nh:
                    tw = cpool.tile([128, nh, nt, 128], f16, tag=f"wh{li}")
                    step = max(1, nh // 4) if li == 2 else max(1, nh // 2)
                    for c0 in range(0, nh, step):
                        nc.gpsimd.dma_start(
                            tw[:, c0 : c0 + step, :, :],
                            wh_d[li][:, c0 : c0 + step, :, :],
                        )
                    whs[li] = tw
                if li in (2, 3):
                    tq = cpool.tile([128, 3, nt], f32, tag=f"wqc{li}")
                    nc.gpsimd.dma_start(tq[:], wqc_d[li][:])
                    wqcs[li] = tq
                if li >= 2:
                    tp = cpool.tile([3, nt, 128], f16, tag=f"wqp{li}")
                    nc.gpsimd.dma_start(tp[:], wqp_d[li][:])
                    wqps[li] = tp
            w6t = cpool.tile([128, 1], f16)
            nc.gpsimd.dma_start(w6t[:], w6_d[:])


            def interp(n):
                """Interpolation weights for chunk n -> codes-major wT."""
                tsq = []
                for a in range(3):
                    tsq_a = wpool.tile([128, NPT, 128], f32, tag=f"tsq{a}",
                                       name=f"tsq{a}")
                    tsq.append(tsq_a)
                for a in range(3):
                    for pt in range(NPT):
                        g = n * NPT + pt
                        # (cb_a - q_a)^2: exact fp32 subtract in the ACT
                        # input stage (bias = -q_a), then Square
                        nc.scalar.activation(
                            tsq[a][:, pt, :], cbt[:, a, :], AF.Square,
                            bias=qptn[:, g, a : a + 1], scale=1.0,
                        )
                s = tsq[0]
                nc.vector.tensor_tensor(s[:], s[:], tsq[1][:], OP.add)
                nc.vector.tensor_tensor(s[:], s[:], tsq[2][:], OP.add)
                nc.vector.tensor_scalar_add(s[:], s[:], 1e-16)
                u = wpool.tile([128, NPT, 128], f32, tag="u")
                nc.vector.reciprocal_approx_fast(out=u[:], in_=s[:])
                dn = wpool.tile([128, NPT], f32, tag="dn")
                nc.vector.tensor_reduce(dn[:], u[:], mybir.AxisListType.X, OP.add)
                rr = wpool.tile([128, NPT], f32, tag="rr")
                nc.vector.reciprocal_approx_fast(out=rr[:], in_=dn[:])
                wts = wpool.tile([128, NPT, 128], f16, tag="wts")
                for pt in range(NPT):
                    nc.vector.tensor_scalar_mul(
                        wts[:, pt, :], u[:, pt, :], rr[:, pt : pt + 1]
                    )
                # transpose w: [points, codes] -> [codes, points]
                wT_ps = psI.tile([128, NPT, 128], f16, tag="wT")
                for pt in range(NPT):
                    nc.tensor.transpose(wT_ps[:, pt, :], wts[:, pt, :], ident[:])
                wT = wpool.tile([128, NPT, 128], f16, tag="wTs")
                nc.scalar.copy(wT[:], wT_ps[:])
                # qp broadcast for the DVE skip-adds of this chunk
                qpb = qpbp.tile([128, 3, TN], f16, tag="qpb")
                nc.sync.dma_start(qpb[:], qpb_d[:, n, :, :])
                return wT, qpb

            state = (wT0, qpb0)
            for _rep in range(REPEAT):
              for n in range(NCHUNK):
                wT, qpb = state

                # ---- L1 (short groups; qp on PE; drains split) ----
                nh, nt = LAYERS[0]
                h_out = hpool.tile([128, nt, TN], f16, tag="h1")
                for m in range(nt):
                    z = psZ.tile([128, TN], f32, tag="z", name=f"z{m % 2}")
                    nc.tensor.matmul(z[:], wms[1][:, m, :], wT[:],
                                     start=True, stop=False)
                    nc.tensor.matmul(z[:], wq1[0:3, m, :],
                                     qp3c[0:3, ts(n, TN)],
                                     start=False, stop=True)
                    if m in (5, 7, 9, 11, 13, 15):
                        # DVE relieves the ACT drain backlog of L1's short
                        # groups (lrelu as copy + max pair; the Pool engine
                        # cannot run stt ops or read PSUM)
                        zc = wpool.tile([128, TN], f16, tag="zc", name="zc")
                        nc.vector.tensor_copy(zc[:], z[:])
                        nc.vector.scalar_tensor_tensor(
                            h_out[:, m, :], zc[:], 0.02, zc[:], OP.mult, OP.max
                        )
                    else:
                        nc.scalar.activation(h_out[:, m, :], z[:], AF.Prelu,
                                             alpha=0.02)
                prev = h_out

                # interp for the NEXT chunk: ACT squares / DVE chain / PE
                # transposes overlap this chunk's deep layers
                if n + 1 < NCHUNK:
                    state = interp(n + 1)

                # ---- L2..L5 (long groups; qp via DVE adds into PSUM) ----
                prev_order = list(range(16))
                for i, (nh, nt) in enumerate(LAYERS[1:], start=2):
                    li = i
                    h_out = hpool.tile([128, nt, TN], f16, tag=f"h{li}")
                    # emit the last DVE-qp tile first so its (slower) drain
                    # chain finishes before the next layer reads it
                    m_order = [nt - 2] + [m for m in range(nt) if m != nt - 2]                         if nt > 2 else list(range(nt))
                    for m in m_order:
                        z = psZ.tile([128, TN], f32, tag="z", name=f"z{m % 2}")
                        pe_qp = li in (4, 5) or m == nt - 1
                        if pe_qp:
                            nc.tensor.matmul(z[:], wqps[li][0:3, m, :],
                                             qp3c[0:3, ts(n, TN)],
                                             start=True, stop=False)
                        for c in range(nh):
                            nc.tensor.matmul(z[:], whs[li][:, c, m, :],
                                             prev[:, c, :],
                                             start=(c == 0 and not pe_qp),
                                             stop=False)
                        nc.tensor.matmul(z[:], wms[li][:, m, :], wT[:],
                                         start=False, stop=True)
                        if not pe_qp:
                            # rank-3 qp skip: z += qp_a * Wq[a, :] (DVE, fp32)
                            for a in range(3):
                                nc.vector.scalar_tensor_tensor(
                                    z[:], qpb[:, a, :],
                                    wqcs[li][:, a, m : m + 1],
                                    z[:], OP.mult, OP.add,
                                )
                        nc.scalar.activation(h_out[:, m, :], z[:], AF.Prelu,
                                             alpha=0.02)
                    prev = h_out
                    prev_order = m_order

                z6 = psS.tile([1, TN], f32, tag="z6")
                nc.tensor.matmul(z6[:], w6t[:], prev[:, 0, :], start=True,
                                 stop=True)
                outb = wpool.tile([1, TN], f32, tag="outb")
                nc.scalar.copy(outb[:], z6[:])
                nc.sync.dma_start(out_d[0:1, ts(n, TN)], outb[:])

    nc.compile()
    return nc


def get_built():
    global _BUILT
    if _BUILT is None:
        _BUILT = _build()
    return _BUILT


def prepare_in_maps(inputs):
    """Host-side gather + packing into per-core input maps."""
    inp = {k: np.asarray(v) for k, v in inputs.items()}
    idx = np.asarray(inp["indices"]).astype(np.int64)
    qp = inp["query_points"].astype(np.float32)
    cp = inp["codes_position"].astype(np.float32)
    codes = inp["codes"].astype(np.float32)

    NHS = [0, 2048, 1024, 512, 256]
    Wl = [inp[f"W{i}"].astype(np.float32) for i in range(1, 7)]

    shared = {}
    for i, (nh, nt) in enumerate(LAYERS):
        li = i + 1
        W = Wl[i]
        if nh:
            shared[f"wh{li}"] = np.ascontiguousarray(
                W[: nh * 128].reshape(nh, 128, nt, 128).transpose(1, 0, 2, 3)
            ).astype(np.float16)
        Wq = W[NHS[i] + 128 :]                    # (3, fout)
        if li == 1:
            shared["wq1"] = np.ascontiguousarray(
                Wq.reshape(3, nt, 128)
            ).astype(np.float16)
        else:
            if li in (2, 3):
                # per-partition scalar columns: [j(128), a, m]
                shared[f"wqc{li}"] = np.ascontiguousarray(
                    Wq.reshape(3, nt, 128).transpose(2, 0, 1)
                ).astype(np.float32)
            shared[f"wqp{li}"] = np.ascontiguousarray(
                Wq.reshape(3, nt, 128)
            ).astype(np.float16)
    shared["w6"] = Wl[5].astype(np.float16)

    in_maps = []
    for b in range(B):
        q = qp[b]                      # (P, 3)
        c = cp[idx[b]]                 # (K, 3)
        bcv = codes[idx[b]].astype(np.float32)
        m = dict(shared)
        for i, (nh, nt) in enumerate(LAYERS):
            li = i + 1
            Wqc = Wl[i][NHS[i] : NHS[i] + 128]     # (128, fout)
            M = (bcv @ Wqc).astype(np.float32)     # codes-space fold
            m[f"wm{li}"] = np.ascontiguousarray(
                M.reshape(128, nt, 128)
            ).astype(np.float16)
        q0 = q[:TN]
        diff0 = q0[:, None, :] - c[None, :, :]
        sd0 = np.sum(diff0 * diff0, axis=-1).astype(np.float32) + 1e-16
        u0 = (1.0 / sd0).astype(np.float32)
        w0 = u0 / u0.sum(-1, keepdims=True)
        m["wt0"] = np.ascontiguousarray(
            w0.reshape(NPT, 128, 128).transpose(2, 0, 1)
        ).astype(np.float16)
        m["qptn"] = np.ascontiguousarray(
            -q.reshape(NTILE_P, 128, 3).transpose(1, 0, 2)
        ).astype(np.float32)
        m["qp3"] = np.ascontiguousarray(q.T).astype(np.float16)
        qb = q.T.reshape(3, NCHUNK, TN).transpose(1, 0, 2)   # (NCHUNK,3,TN)
        m["qpb"] = np.ascontiguousarray(
            np.broadcast_to(qb[None], (128, NCHUNK, 3, TN))
        ).astype(np.float16)
        m["cb"] = np.ascontiguousarray(
            np.broadcast_to(c.T[None, :, :], (128, 3, K))
        ).astype(np.float32)
        in_maps.append(m)
    return in_maps


def run(inputs, trace=False, **kw):
    nc = get_built()
    in_maps = prepare_in_maps(inputs)
    res = run_bass_kernel_spmd(nc, in_maps, core_ids=list(range(B)), trace=trace, **kw)
    out = np.concatenate([np.asarray(r["out"]) for r in res.results], axis=0)
    return out.astype(np.float32), res


def kernel(**inputs):
    out, _ = run(inputs, trace=False)
    return out
